# revision 6
# baseline (speedup 1.0000x reference)
"""Trainium2 Bass kernel for the AttentionModule problem (v5).

Cross-attention with normalized-position RoPE:
  q = Wq @ x;  k = Wk @ ctx;  v = Wv @ ctx  (per-head RoPE on q, k)
  out = Wo @ (softmax(q^T k / sqrt(512)) @ v), masked.

Sharding: 8 cores = 4 batches x 2 T-halves, collective-free.

v5 pipeline (Scalar exp = the hard floor: 128 ACTs x ~1.1us):
  phase A: Q/K projections (single projection + PE permutation matmul
    for the rotate-half term -- no twin weights) woven with head-pair
    0's S+exp; proj psum and S psum share one pool.
  window 1: S/exp of hp1 + V projection (psum shared with the S-tile
    rotation, DVE drains) + O of hp0 (e-tiles stayed resident).
  windows 2,3: S/exp of hp + O of hp-1.
  tail: O of hp3 woven with phase-5 tg0 matmuls, normalize via
    reciprocal_approx_fast, remaining phase-5, stores.
  Input DMAs issue from four engine queues in parallel.
"""

import math
import sys
import types

sys.path.insert(0, "/opt/trn_rl_repo")

import numpy as np
import ml_dtypes

import concourse.bass as bass
import concourse.tile as tile
from concourse import bacc, mybir
from concourse.bass_utils import run_bass_kernel_spmd

D_MODEL = 512
D_CONTEXT = 512
NUM_HEADS = 8
ATTN_DIM = 512
HEAD_DIM = 64
ROPE_GAMMA = 10.0
ATTN_SCALE = math.sqrt(ATTN_DIM)
B = 4
T_FULL = 2048
L = 2048
N_CORES = 8
T = T_FULL // 2
P = 128
NAC = ATTN_DIM // P
NLC = L // P
FP32 = mybir.dt.float32
BF16 = mybir.dt.bfloat16
AF = mybir.ActivationFunctionType
ALU = mybir.AluOpType

_GRAPH_CACHE = {}


def _ensure_ntff_hook():
    if "antenv.axon_hooks" in sys.modules:
        return
    try:
        mod = types.ModuleType("antenv.axon_hooks")
        mod._hook = None
        mod.set_axon_ntff_profile_hook = lambda h: setattr(mod, "_hook", h)
        mod.get_axon_ntff_profile_hook = lambda: mod._hook
        sys.modules["antenv.axon_hooks"] = mod
        from trn_agent_boot.trn_boot import _ntff_profile_via_ctypes

        mod.set_axon_ntff_profile_hook(
            _ntff_profile_via_ctypes("/opt/axon/libaxon_pjrt.so")
        )
    except Exception:
        pass


def _build_graph(use_bias: bool, use_cmask: bool, use_xmask: bool):
    nc = bacc.Bacc("TRN2", target_bir_lowering=False, debug=False, num_devices=N_CORES)

    x_d = nc.dram_tensor("x", [D_MODEL, T], BF16, kind="ExternalInput").ap()
    ctx_d = nc.dram_tensor("ctxT", [D_CONTEXT, L], BF16, kind="ExternalInput").ap()
    w_d = {
        name: nc.dram_tensor(name, [512, 512], BF16, kind="ExternalInput").ap()
        for name in ("wq", "wk", "wv", "wo")
    }
    perm_d = nc.dram_tensor("perm", [P, P], BF16, kind="ExternalInput").ap()
    cosq_d = nc.dram_tensor("cosq", [P, T], BF16, kind="ExternalInput").ap()
    sinq_d = nc.dram_tensor("sinq", [P, T], BF16, kind="ExternalInput").ap()
    cosk_d = nc.dram_tensor("cosk", [P, L], BF16, kind="ExternalInput").ap()
    sink_d = nc.dram_tensor("sink", [P, L], BF16, kind="ExternalInput").ap()
    if use_bias:
        bias_d = nc.dram_tensor("biases", [6, 512], BF16, kind="ExternalInput").ap()
    if use_cmask:
        logcm_d = nc.dram_tensor("logcm", [P, NLC], FP32, kind="ExternalInput").ap()
    if use_xmask:
        xmask_d = nc.dram_tensor("xmaskb", [P, T], FP32, kind="ExternalInput").ap()
    out_d = nc.dram_tensor("out", [D_MODEL, T], FP32, kind="ExternalOutput").ap()

    inv_scale = 1.0 / ATTN_SCALE

    with tile.TileContext(nc) as tc:
        with (
            tc.tile_pool(name="const", bufs=1) as const,
            tc.tile_pool(name="big", bufs=1) as big,
            tc.tile_pool(name="nrm", bufs=1) as nrm_pool,
            tc.tile_pool(name="oraw", bufs=1) as oraw_pool,
            tc.tile_pool(name="outp", bufs=2) as out_pool,
            tc.tile_pool(name="epool", bufs=1) as e_pool,
            tc.tile_pool(name="tmp", bufs=2) as tmp_pool,
        ):
            # ---- input DMAs in need-order, spread across engine queues ----
            w_bf = {}
            issuers = [nc.sync, nc.gpsimd]
            _issue_i = [0]

            def dma(dst, src):
                eng = issuers[_issue_i[0] % len(issuers)]
                _issue_i[0] += 1
                eng.dma_start(dst, src)

            def load_weight(name):
                wt = big.tile([P, NAC, 512], BF16, tag=f"w_{name}", name=f"w_{name}")
                dma(wt[:], w_d[name].rearrange("(c p) a -> p c a", p=P))
                w_bf[name] = wt

            perm_sb = const.tile([P, P], BF16)
            dma(perm_sb[:], perm_d[:])
            # first wave: exactly what Q(0,0..1)/K(0,0..1) + S(hp0,lc0..7)
            # need, in small pieces so the first exp isn't gated on bulk
            x_bf = big.tile([P, NAC, T], BF16)
            wq_t = big.tile([P, NAC, 512], BF16, tag="w_wq", name="w_wq")
            w_bf["wq"] = wq_t
            wk_t = big.tile([P, NAC, 512], BF16, tag="w_wk", name="w_wk")
            w_bf["wk"] = wk_t
            ctx_bf = big.tile([P, NAC, L], BF16)
            cos_q = const.tile([P, T], BF16)
            sin_q = const.tile([P, T], BF16)
            cos_k = const.tile([P, L], BF16)
            sin_k = const.tile([P, L], BF16)
            dma(wq_t[:, :, 0:P], w_d["wq"][:, 0:P].rearrange("(c p) a -> p c a", p=P))
            dma(x_bf[:, :, 0:512], x_d[:, 0:512].rearrange("(c p) a -> p c a", p=P))
            dma(cos_q[:], cosq_d[:])
            dma(sin_q[:], sinq_d[:])
            dma(x_bf[:, :, 512:1024], x_d[:, 512:1024].rearrange("(c p) a -> p c a", p=P))
            dma(wk_t[:, :, 0:P], w_d["wk"][:, 0:P].rearrange("(c p) a -> p c a", p=P))
            dma(ctx_bf[:, :, 0:1024], ctx_d[:, 0:1024].rearrange("(c p) a -> p c a", p=P))
            dma(cos_k[:, 0:1024], cosk_d[:, 0:1024])
            dma(sin_k[:, 0:1024], sink_d[:, 0:1024])
            # second wave
            dma(wq_t[:, :, P:512], w_d["wq"][:, P:512].rearrange("(c p) a -> p c a", p=P))
            dma(wk_t[:, :, P:512], w_d["wk"][:, P:512].rearrange("(c p) a -> p c a", p=P))
            dma(ctx_bf[:, :, 1024:2048], ctx_d[:, 1024:2048].rearrange("(c p) a -> p c a", p=P))
            dma(cos_k[:, 1024:2048], cosk_d[:, 1024:2048])
            dma(sin_k[:, 1024:2048], sink_d[:, 1024:2048])
            load_weight("wv")
            load_weight("wo")
            zero_b = const.tile([P, 1], FP32)
            nc.vector.memset(zero_b[:], 0.0)
            if use_cmask:
                logcm_sb = const.tile([P, NLC], FP32)
                dma(logcm_sb[:], logcm_d[:])
            if use_xmask:
                xmask_sb = const.tile([P, T], FP32)
                dma(xmask_sb[:], xmask_d[:])
            if use_bias:
                bias_bf = const.tile([1, 6, 512], BF16)
                dma(bias_bf[:], bias_d.rearrange("b a -> 1 b a"))
                ones_row = const.tile([1, 512], BF16)
                nc.vector.memset(ones_row[:], 1.0)
                ones_col = const.tile([1, P], BF16)
                nc.vector.memset(ones_col[:], 1.0)

            q_rope = big.tile([P, NAC, T], BF16)
            k_rope = big.tile([P, NAC, L], BF16)
            v1 = big.tile([P, NLC, NUM_HEADS, HEAD_DIM + 1], BF16)
            nc.vector.memset(v1[:, :, :, HEAD_DIM : HEAD_DIM + 1], 1.0)
            o_norm = big.tile([P, NAC, T], BF16)

            e_tiles = {}  # (hp, h01, lc) -> tile

            def alloc_e(hp, h01, lc):
                t = e_pool.tile([P, T], BF16, tag=f"e{h01}", bufs=18, name=f"e_{h01}")
                e_tiles[(hp, h01, lc)] = t
                return t

            deferred = []

            def do_normalize(*pairs):
                # batched over up to 2 heads: all DVE den/recip first, then the
                # GpSimd broadcasts, then the multiplies -- so the in-order DVE
                # queue never blocks on a GpSimd broadcast mid-chain
                stage = []
                for o_raw, h in pairs:
                    den = nrm_pool.tile([1, T], FP32, tag="den", bufs=1)
                    nc.vector.tensor_copy(den[:], o_raw[HEAD_DIM : HEAD_DIM + 1, :])
                    rec = nrm_pool.tile([1, T], FP32, tag="rec", bufs=2)
                    nc.vector.reciprocal_approx_fast(rec[:], den[:])
                    stage.append((o_raw, h, rec))
                stage2 = []
                for o_raw, h, rec in stage:
                    rb = nrm_pool.tile([64, T], FP32, tag="rb", bufs=2)
                    nc.gpsimd.partition_broadcast(rb[:], rec[:], channels=64)
                    stage2.append((o_raw, h, rb))
                for o_raw, h, rb in stage2:
                    r0 = (h % 2) * 64
                    nc.vector.tensor_tensor(
                        o_norm[r0 : r0 + 64, h // 2, :],
                        o_raw[0:HEAD_DIM, :],
                        rb[:],
                        op=ALU.mult,
                    )

            def emit_S(pool, hp, lc, e_dst_a, e_dst_b):
                s_a = pool.tile([P, T], FP32, tag="s", bufs=2, name="s_a")
                s_b = pool.tile([P, T], FP32, tag="s", bufs=2, name="s_b")
                for tg in range(T // 512):
                    sl = slice(tg * 512, (tg + 1) * 512)
                    for rows, s_t in ((slice(0, 64), s_a), (slice(64, 128), s_b)):
                        nc.tensor.matmul(
                            s_t[:, sl],
                            lhsT=k_rope[rows, hp, lc * P : (lc + 1) * P],
                            rhs=q_rope[rows, hp, sl],
                            start=True,
                            stop=True,
                        )
                eb = logcm_sb[:, lc : lc + 1] if use_cmask else zero_b[:]
                nc.scalar.activation(e_dst_a[:], s_a[:], AF.Exp, bias=eb, scale=inv_scale)
                nc.scalar.activation(e_dst_b[:], s_b[:], AF.Exp, bias=eb, scale=inv_scale)

            def emit_O_q4(hp, q4, po_a, po_b):
                for lc4 in range(2):
                    lc = q4 * 2 + lc4
                    for h01, po in ((0, po_a), (1, po_b)):
                        for tg in range(T // 512):
                            sl = slice(tg * 512, (tg + 1) * 512)
                            nc.tensor.matmul(
                                po[:, sl],
                                lhsT=v1[:, lc, 2 * hp + h01, :],
                                rhs=e_tiles[(hp, h01, lc)][:, sl],
                                start=(lc == 0),
                                stop=(lc == NLC - 1),
                            )

            def drain_po(hp, po_a, po_b):
                for ps, h, tg_ in ((po_a, 2 * hp, "orA"), (po_b, 2 * hp + 1, "orB")):
                    o_raw = oraw_pool.tile([HEAD_DIM + 1, T], FP32, tag=tg_, name="o_raw")
                    nc.vector.tensor_copy(o_raw[:], ps[:])
                    deferred.append((o_raw, h))
                for k in [k for k in e_tiles if k[0] == hp]:
                    del e_tiles[k]
                if len(deferred) > 3:
                    do_normalize(deferred.pop(0), deferred.pop(0))
                while len(deferred) > 2:
                    do_normalize(deferred.pop(0))

            # projection tasks are software-pipelined: the perm matmul of
            # task n-1 is emitted after task n's projection matmuls so the PE
            # never waits on the DVE sigma-multiply.
            proj_pending = []

            def proj_flush(pool):
                while proj_pending:
                    ps, tt, out_tile, cos_t, ac, sl = proj_pending.pop(0)
                    ttp = pool.tile([P, 512], FP32, tag="pj", bufs=3, name="ttp")
                    nc.tensor.matmul(
                        ttp[:], lhsT=perm_sb[:], rhs=tt[:], start=True, stop=True
                    )
                    nc.vector.tensor_tensor(
                        out_tile[:, ac, sl], ps[:], cos_t[:, sl], op=ALU.mult
                    )
                    nc.vector.tensor_tensor(
                        out_tile[:, ac, sl], ttp[:], out_tile[:, ac, sl], op=ALU.add
                    )

            def proj_task(pool, wn, rhs_tile, out_tile, cos_t, sin_t, bq, ac, g):
                # single projection + PE permutation for the rotate-half term
                sl = slice(g * 512, (g + 1) * 512)
                ps = pool.tile([P, 512], FP32, tag="pj", bufs=3, name="ps_pj")
                for dc in range(NAC):
                    nc.tensor.matmul(
                        ps[:],
                        lhsT=w_bf[wn][:, dc, ac * P : (ac + 1) * P],
                        rhs=rhs_tile[:, dc, sl],
                        start=(dc == 0),
                        stop=(dc == NAC - 1) and not use_bias,
                    )
                if use_bias:
                    nc.tensor.matmul(
                        ps[:],
                        lhsT=bias_bf[:, bq, ac * P : (ac + 1) * P],
                        rhs=ones_row[:],
                        start=False,
                        stop=True,
                    )
                tt = tmp_pool.tile([P, 512], BF16, tag="ropetmp")
                nc.vector.tensor_tensor(tt[:], ps[:], sin_t[:, sl], op=ALU.mult)
                proj_pending.append((ps, tt, out_tile, cos_t, ac, sl))

            def v_task(pool, lc):
                ps_v = pool.tile([P, 512], FP32, tag="psv", bufs=1, name="ps_v")
                ps_v = ps_v[:]
                for dc in range(NAC):
                    nc.tensor.matmul(
                        ps_v,
                        lhsT=ctx_bf[:, dc, lc * P : (lc + 1) * P],
                        rhs=w_bf["wv"][:, dc, :],
                        start=(dc == 0),
                        stop=(dc == NAC - 1) and not use_bias,
                    )
                if use_bias:
                    nc.tensor.matmul(
                        ps_v,
                        lhsT=ones_col[:],
                        rhs=bias_bf[:, 4, :],
                        start=False,
                        stop=True,
                    )
                if lc < 10:
                    # Scalar drain while DVE is saturated with rope combines
                    nc.scalar.activation(
                        v1[:, lc, :, 0:HEAD_DIM],
                        ps_v.rearrange("p (h d) -> p h d", d=HEAD_DIM),
                        AF.Identity,
                        bias=zero_b[:],
                    )
                else:
                    nc.vector.tensor_copy(
                        v1[:, lc, :, 0:HEAD_DIM],
                        ps_v.rearrange("p (h d) -> p h d", d=HEAD_DIM),
                    )

            # ---- phase A: Q/K projections + hp0 S/exp ----
            with tc.tile_pool(name="psA", bufs=1, space="PSUM") as psA:
                warm = psA.tile([P, 512], FP32, tag="pj", bufs=3, name="warm")
                for i in range(14):
                    nc.tensor.matmul(
                        warm[:],
                        lhsT=perm_sb[:],
                        rhs=cos_q[:, 0:512],
                        start=(i == 0),
                        stop=(i == 13),
                    )

                def Q(ac, g):
                    proj_task(psA, "wq", x_bf, q_rope, cos_q, sin_q, 0, ac, g)

                def K(ac, g):
                    proj_task(psA, "wk", ctx_bf, k_rope, cos_k, sin_k, 2, ac, g)

                Q(0, 0)
                K(0, 0)
                Q(0, 1)
                K(0, 1)
                proj_flush(psA)
                rest = [(K, 0, 2), (K, 0, 3)]
                for ac in (1, 2, 3):
                    rest.append((Q, ac, 0))
                    rest.append((Q, ac, 1))
                    for g in range(4):
                        rest.append((K, ac, g))
                ri = 0
                for lc in range(NLC):
                    v_task(psA, lc)
                    for _ in range(2 if lc % 3 == 0 else 1):
                        if ri < len(rest):
                            f, ac, g = rest[ri]
                            proj_flush(psA)
                            f(ac, g)
                            ri += 1
                    e_a = alloc_e(0, 0, lc)
                    e_b = alloc_e(0, 1, lc)
                    emit_S(psA, 0, lc, e_a, e_b)
                while ri < len(rest):
                    f, ac, g = rest[ri]
                    proj_flush(psA)
                    f(ac, g)
                    ri += 1
                proj_flush(psA)

            # ---- phase B ----
            with tc.tile_pool(name="psO", bufs=1, space="PSUM") as psO:
                with tc.tile_pool(name="psS", bufs=1, space="PSUM") as psS:
                    po3 = None
                    for hp in (1, 2):
                        po_a = psO.tile(
                            [HEAD_DIM + 1, T], FP32, tag="po", bufs=2, name="po_a"
                        )
                        po_b = psO.tile(
                            [HEAD_DIM + 1, T], FP32, tag="po", bufs=2, name="po_b"
                        )
                        for q4 in range(8):
                            emit_O_q4(hp - 1, q4, po_a, po_b)
                            for lc4 in range(2):
                                lc = 2 * q4 + lc4
                                e_a = alloc_e(hp, 0, lc)
                                e_b = alloc_e(hp, 1, lc)
                                emit_S(psS, hp, lc, e_a, e_b)
                        drain_po(hp - 1, po_a, po_b)
                    # window 3: O(hp2) compressed into the first half; O(hp3)
                    # runs pending-style in the second half (po pairs never
                    # coexist), leaving only O(3, q4=6,7) for the tail
                    po_a = psO.tile(
                        [HEAD_DIM + 1, T], FP32, tag="po", bufs=2, name="po_a"
                    )
                    po_b = psO.tile(
                        [HEAD_DIM + 1, T], FP32, tag="po", bufs=2, name="po_b"
                    )
                    for q4 in range(8):
                        if q4 < 4:
                            emit_O_q4(2, 2 * q4, po_a, po_b)
                            emit_O_q4(2, 2 * q4 + 1, po_a, po_b)
                        else:
                            if q4 == 4:
                                drain_po(2, po_a, po_b)
                                po3 = (
                                    psO.tile(
                                        [HEAD_DIM + 1, T], FP32, tag="po",
                                        bufs=2, name="po_a",
                                    ),
                                    psO.tile(
                                        [HEAD_DIM + 1, T], FP32, tag="po",
                                        bufs=2, name="po_b",
                                    ),
                                )
                            else:
                                emit_O_q4(3, 2 * (q4 - 5), po3[0], po3[1])
                                emit_O_q4(3, 2 * (q4 - 5) + 1, po3[0], po3[1])
                        for lc4 in range(2):
                            lc = 2 * q4 + lc4
                            e_a = alloc_e(3, 0, lc)
                            e_b = alloc_e(3, 1, lc)
                            emit_S(psS, 3, lc, e_a, e_b)
                # psS closed: its 4 banks host phase-5 tg0 groups during the
                # hp3 O burst
                with tc.tile_pool(name="ps5a", bufs=1, space="PSUM") as ps5a:
                    po5 = {}
                    for dmc in range(NAC):
                        po5[(0, dmc)] = ps5a.tile(
                            [P, 512], FP32, tag=f"p50_{dmc}", name=f"po50_{dmc}"
                        )
                    # heads 4,5 were drained mid-window-3: normalize them
                    # now so ph5 ac0-2 can weave with the remaining O work
                    while deferred:
                        if len(deferred) >= 2:
                            do_normalize(deferred.pop(0), deferred.pop(0))
                        else:
                            do_normalize(deferred.pop(0))
                    for q4 in (6, 7):
                        emit_O_q4(3, q4, po3[0], po3[1])
                    for j in range(12):
                        ac, dmc = divmod(j, NAC)
                        nc.tensor.matmul(
                            po5[(0, dmc)][:, :],
                            lhsT=w_bf["wo"][:, ac, dmc * P : (dmc + 1) * P],
                            rhs=o_norm[:, ac, 0:512],
                            start=(ac == 0),
                            stop=False,
                        )
                    drain_po(3, po3[0], po3[1])
                    while deferred:
                        if len(deferred) >= 2:
                            do_normalize(deferred.pop(0), deferred.pop(0))
                        else:
                            do_normalize(deferred.pop(0))
                    # finish tg0: ac=3 for all dmc
                    for ac in (3,):
                        for dmc in range(NAC):
                            nc.tensor.matmul(
                                po5[(0, dmc)][:],
                                lhsT=w_bf["wo"][:, ac, dmc * P : (dmc + 1) * P],
                                rhs=o_norm[:, ac, 0:512],
                                start=False,
                                stop=(ac == 3) and not use_bias,
                            )
                    for dmc in range(NAC):
                        if use_bias:
                            nc.tensor.matmul(
                                po5[(0, dmc)][:],
                                lhsT=bias_bf[:, 5, dmc * P : (dmc + 1) * P],
                                rhs=ones_row[:],
                                start=False,
                                stop=True,
                            )
                        ot = out_pool.tile([P, 512], FP32, tag="ot")
                        if use_xmask:
                            nc.vector.tensor_tensor(
                                ot[:], po5[(0, dmc)][:], xmask_sb[:, 0:512], op=ALU.mult
                            )
                        else:
                            nc.vector.tensor_copy(ot[:], po5[(0, dmc)][:])
                        dma(out_d[dmc * P : (dmc + 1) * P, 0:512], ot[:])

            # tg=1 after psO closes
            with tc.tile_pool(name="ps5b", bufs=1, space="PSUM") as ps5b:
                sl = slice(512, 1024)
                po51 = {}
                for dmc in range(NAC):
                    po51[dmc] = ps5b.tile(
                        [P, 512], FP32, tag=f"p51_{dmc}", name=f"po51_{dmc}"
                    )
                for ac in range(NAC):
                    for dmc in range(NAC):
                        nc.tensor.matmul(
                            po51[dmc][:],
                            lhsT=w_bf["wo"][:, ac, dmc * P : (dmc + 1) * P],
                            rhs=o_norm[:, ac, sl],
                            start=(ac == 0),
                            stop=(ac == NAC - 1) and not use_bias,
                        )
                for dmc in range(NAC):
                    po = po51[dmc]
                    if use_bias:
                        nc.tensor.matmul(
                            po[:],
                            lhsT=bias_bf[:, 5, dmc * P : (dmc + 1) * P],
                            rhs=ones_row[:],
                            start=False,
                            stop=True,
                        )
                    ot = out_pool.tile([P, 512], FP32, tag="ot")
                    if use_xmask:
                        nc.vector.tensor_tensor(
                            ot[:], po[:], xmask_sb[:, sl], op=ALU.mult
                        )
                    else:
                        nc.vector.tensor_copy(ot[:], po[:])
                    dma(out_d[dmc * P : (dmc + 1) * P, sl], ot[:])

    nc.compile()
    return nc


def kernel(
    x,
    context,
    x_mask,
    context_mask,
    Wq_w,
    Wq_b,
    Wk_w,
    Wk_b,
    Wv_w,
    Wv_b,
    Wo_w,
    Wo_b,
    _want_trace=False,
):
    _ensure_ntff_hook()
    bf = ml_dtypes.bfloat16
    x = np.asarray(x, np.float32)
    context = np.asarray(context, np.float32)
    x_mask = np.asarray(x_mask, np.float32)
    context_mask = np.asarray(context_mask, np.float32)
    weights = {
        "wq": np.ascontiguousarray(np.asarray(Wq_w, np.float32).T.astype(bf)),
        "wk": np.ascontiguousarray(np.asarray(Wk_w, np.float32).T.astype(bf)),
        "wv": np.ascontiguousarray(np.asarray(Wv_w, np.float32).T.astype(bf)),
        "wo": np.ascontiguousarray(np.asarray(Wo_w, np.float32).T.astype(bf)),
    }
    biases = np.stack(
        [
            np.asarray(Wq_b, np.float32),
            np.zeros(512, np.float32),
            np.asarray(Wk_b, np.float32),
            np.zeros(512, np.float32),
            np.asarray(Wv_b, np.float32),
            np.asarray(Wo_b, np.float32),
        ]
    )

    use_bias = bool(np.any(biases != 0.0))
    use_cmask = not bool(np.all(context_mask == 1.0))
    use_xmask = not bool(np.all(x_mask == 1.0))

    key = (use_bias, use_cmask, use_xmask)
    if key not in _GRAPH_CACHE:
        _GRAPH_CACHE[key] = _build_graph(*key)
    nc = _GRAPH_CACHE[key]

    # permutation matrix: out[m] = in[pair(m)], pair flips +-32 within each
    # 64-row head block
    perm = np.zeros((P, P), np.float32)
    for m in range(P):
        pm = m + 32 if (m % 64) < 32 else m - 32
        perm[pm, m] = 1.0

    len_q = x_mask.sum(axis=(1, 2))
    len_k = context_mask.sum(axis=(1, 2))
    theta = (1.0 / (10000.0 ** (np.arange(32, dtype=np.float64) / 32.0))) * ROPE_GAMMA
    theta128 = np.tile(theta, 4)[:, None]
    # sign-baked sin (sigma): upper half of each 64-block negated
    sgn = np.where((np.arange(P) % 64) < 32, 1.0, -1.0)[:, None]

    xbf = x.astype(bf)
    ctxbf = np.ascontiguousarray(np.transpose(context, (0, 2, 1))).astype(bf)

    in_maps = []
    for c in range(N_CORES):
        b, th = c // 2, c % 2
        t0 = th * T
        pos_q = (np.arange(t0, t0 + T, dtype=np.float64)[None, :]) / len_q[b]
        pos_k = (np.arange(L, dtype=np.float64)[None, :]) / len_k[b]
        fq = theta128 * pos_q
        fk = theta128 * pos_k
        m = {
            "x": np.ascontiguousarray(xbf[b, :, t0 : t0 + T]),
            "ctxT": ctxbf[b],
            "perm": perm.astype(bf),
            "cosq": np.cos(fq).astype(bf),
            "sinq": (np.sin(fq) * sgn).astype(bf),
            "cosk": np.cos(fk).astype(bf),
            "sink": (np.sin(fk) * sgn).astype(bf),
            **weights,
        }
        if use_bias:
            m["biases"] = biases.astype(bf)
        if use_cmask:
            with np.errstate(divide="ignore"):
                lcm = np.log(context_mask[b, 0]).astype(np.float32)
            m["logcm"] = np.ascontiguousarray(lcm.reshape(NLC, P).T)
        if use_xmask:
            m["xmaskb"] = np.ascontiguousarray(
                np.broadcast_to(x_mask[b, 0, t0 : t0 + T], (P, T))
            )
        in_maps.append(m)

    res = run_bass_kernel_spmd(
        nc, in_maps, core_ids=list(range(N_CORES)), trace=_want_trace
    )
    out = np.empty((B, D_MODEL, T_FULL), np.float32)
    for c in range(N_CORES):
        b, th = c // 2, c % 2
        out[b, :, th * T : (th + 1) * T] = res.results[c]["out"]
    if _want_trace:
        return out, res
    return out
